# revision 20
# baseline (speedup 1.0000x reference)
"""GNN decoder (grid<-mesh message passing) as a Bass/Tile kernel on 8 TRN2 NeuronCores.

Strategy (graph/data parallel per the sharding hint):
  - Grid nodes sharded 2048/core; each edge lives on its receiver's core, sorted by
    receiver and padded per 128-node block so scatter-add is a local one-hot matmul.
  - Mesh table + all weights replicated.
  - Edge message MLP first layer decomposed: cat(x_i,x_j,e) @ W1 =
    (grid@W1a)[recv] + (mesh@W1b)[send] + e@W1c  -> per-node precompute + row gathers.
  - LayerNorm affine/g/beta and all biases folded into weights / evac bias on host;
    LN itself = bn_stats on PSUM + normalize fused into the PSUM->SBUF evacuation.
  - All matmuls in float32r (full PE rate, ~1e-4 rel err).
"""
import os
import sys
import types

import numpy as np

_TRN_REPO = "/opt/trn_rl_repo"
if _TRN_REPO not in sys.path:
    sys.path.append(_TRN_REPO)

import concourse.bass as bass
import concourse.tile as tile
from concourse import mybir
from concourse.bass_utils import run_bass_kernel_spmd

F32 = mybir.dt.float32
F32R = mybir.dt.float32r
I32 = mybir.dt.int32
AF = mybir.ActivationFunctionType
OP = mybir.AluOpType

N_MESH, N_GRID, N_EDGE = 2562, 16384, 49152
D, EDGE_DIM, OUT_DIM = 512, 4, 128
LN_EPS = 1e-5
NC_ = 8                      # cores
NG = N_GRID // NC_           # 2048 grid nodes per core
NB = NG // 128               # 16 blocks per core
MESH_PAD = 2688              # 21 * 128
P = 128


def _fix_multi_waits(nc, max_waits=1):
    """walrus setupSyncWait rejects >~2 sem waits per instruction; hoist extras
    onto preceding same-engine no-op carriers (cheap, unlike a GpSimd DRAIN)."""
    for f in nc.m.functions:
        for b in f.blocks:
            new_insts = []
            for inst in b.instructions:
                si = inst.sync_info
                if si is not None and len(si.on_wait) > max_waits:
                    waits = list(si.on_wait)
                    extra, keep = waits[:-max_waits], waits[-max_waits:]
                    for i in range(0, len(extra), max_waits):
                        d = mybir.InstEventSemaphore(
                            name=nc.get_next_instruction_name(),
                            ins=[], outs=[])
                        d.engine = inst.engine
                        d.sync_info = mybir.SyncInfo(
                            on_wait=extra[i:i + max_waits], on_update=[])
                        nc.register_instruction(d, overwrite=True)
                        new_insts.append(d)
                    inst.sync_info = mybir.SyncInfo(
                        on_wait=keep, on_update=list(si.on_update))
                new_insts.append(inst)
            b.instructions = new_insts


def _install_ntff_shim():
    """This image's antenv lacks axon_hooks; shim it so trace=True works."""
    if 'antenv.axon_hooks' in sys.modules:
        return
    mod = types.ModuleType('antenv.axon_hooks')
    _hook = [None]
    mod.set_axon_ntff_profile_hook = lambda h: _hook.__setitem__(0, h)
    mod.get_axon_ntff_profile_hook = lambda: _hook[0]
    sys.modules['antenv.axon_hooks'] = mod
    try:
        import antenv
        antenv.axon_hooks = mod
        from trn_agent_boot.trn_boot import _ntff_profile_via_ctypes
        mod.set_axon_ntff_profile_hook(
            _ntff_profile_via_ctypes('/opt/axon/libaxon_pjrt.so'))
    except Exception:
        pass


def _ln_evac(nc, sb, ps_in, out_ap, p, eps_tile, free, tag):
    """out = LN(ps_in) (no affine): bn_stats on PSUM, normalize during evacuation."""
    stats = sb.tile([P, 6], F32, tag=f"st_{tag}", name=f"st_{tag}")
    nc.vector.bn_stats(out=stats[:p, :], in_=ps_in[:p, :free])
    mv = sb.tile([P, 2], F32, tag=f"mv_{tag}", name=f"mv_{tag}")
    nc.vector.bn_aggr(out=mv[:p, :], in_=stats[:p, :])
    rstd = sb.tile([P, 1], F32, tag=f"rs_{tag}", name=f"rs_{tag}")
    nc.scalar.activation(out=rstd[:p], in_=mv[:p, 1:2], func=AF.Sqrt,
                         bias=eps_tile[:p], scale=1.0)
    nc.vector.reciprocal(out=rstd[:p], in_=rstd[:p])
    negmr = sb.tile([P, 1], F32, tag=f"nm_{tag}", name=f"nm_{tag}")
    nc.vector.tensor_scalar(out=negmr[:p], in0=mv[:p, 0:1], scalar1=rstd[:p],
                            scalar2=-1.0, op0=OP.mult, op1=OP.mult)
    nc.scalar.activation(out=out_ap, in_=ps_in[:p, :free], func=AF.Identity,
                         bias=negmr[:p], scale=rstd[:p])


def _build(c_blk, flags, mm_dt=None):
    """Build the SPMD Bass module for per-block edge capacity c_blk."""
    F32R = mm_dt if mm_dt is not None else mybir.dt.float32r
    (has_b2_e, has_b2_g, has_b2_n, has_b2_f, has_deg, has_fin_aff, has_b1row) = flags
    e_pad = NB * c_blk
    tiles_per_blk = [(t * P, min(P, c_blk - t * P))
                     for t in range((c_blk + P - 1) // P)]
    CH = 256                      # node-chunk width in phase C

    nc = bass.Bass()
    # ---- per-core inputs ----
    xgT_t = nc.dram_tensor("xgT", [P, 4, NG], F32R, kind="ExternalInput")
    attrT_t = nc.dram_tensor("attrT", [EDGE_DIM, e_pad], F32R, kind="ExternalInput")
    send_t = nc.dram_tensor("send", [e_pad, 1], I32, kind="ExternalInput")
    recvl_t = nc.dram_tensor("recvl", [e_pad, 1], I32, kind="ExternalInput")
    recvb_t = nc.dram_tensor("recvb", [e_pad, 1], F32, kind="ExternalInput")
    # ---- replicated inputs ----
    meshT_t = nc.dram_tensor("meshT", [P, 4, MESH_PAD], F32R, kind="ExternalInput")
    ident_t = nc.dram_tensor("ident", [P, P], F32R, kind="ExternalInput")
    ones_t = nc.dram_tensor("ones", [1, P], F32R, kind="ExternalInput")
    w_h1_t = nc.dram_tensor("w_h1", [EDGE_DIM, D], F32R, kind="ExternalInput")
    b_h1_t = nc.dram_tensor("b_h1", [P, 4], F32, kind="ExternalInput")
    w_h2_t = nc.dram_tensor("w_h2", [P, 4, D], F32R, kind="ExternalInput")
    w_ag_t = nc.dram_tensor("w_ag", [P, 4, D], F32R, kind="ExternalInput")
    w_am_t = nc.dram_tensor("w_am", [P, 4, D], F32R, kind="ExternalInput")
    w_ce_t = nc.dram_tensor("w_ce", [P, 4, D], F32R, kind="ExternalInput")
    w_g2_t = nc.dram_tensor("w_g2", [P, 4, D], F32R, kind="ExternalInput")
    b1row_t = nc.dram_tensor("b1row", [1, D], F32R, kind="ExternalInput")
    w_n1_t = nc.dram_tensor("w_n1", [P, 8, D], F32R, kind="ExternalInput")
    b_n1_t = nc.dram_tensor("b_n1", [P, 4], F32, kind="ExternalInput")
    w_n2_t = nc.dram_tensor("w_n2", [P, 4, D], F32R, kind="ExternalInput")
    w_f1_t = nc.dram_tensor("w_f1", [P, 8, D], F32R, kind="ExternalInput")
    b_f1_t = nc.dram_tensor("b_f1", [P, 4], F32, kind="ExternalInput")
    w_f2_t = nc.dram_tensor("w_f2", [P, 4, 256], F32R, kind="ExternalInput")
    # optional generality inputs (always declared; tiny)
    b2e_t = nc.dram_tensor("b2e", [1, D], F32R, kind="ExternalInput")
    b2g_t = nc.dram_tensor("b2g", [1, D], F32R, kind="ExternalInput")
    b2n_t = nc.dram_tensor("b2n", [1, D], F32R, kind="ExternalInput")
    b2f_t = nc.dram_tensor("b2f", [1, 256], F32R, kind="ExternalInput")
    deg_t = nc.dram_tensor("deg", [1, NG], F32R, kind="ExternalInput")
    cdeg_t = nc.dram_tensor("cdeg", [1, D], F32R, kind="ExternalInput")
    fing_t = nc.dram_tensor("fing", [P, OUT_DIM], F32, kind="ExternalInput")
    finb_t = nc.dram_tensor("finb", [P, OUT_DIM], F32, kind="ExternalInput")

    out_t = nc.dram_tensor("out", [NG, OUT_DIM], F32, kind="ExternalOutput")
    agdram = nc.dram_tensor("agdram", [NG, D], F32R)
    amdram = nc.dram_tensor("amdram", [MESH_PAD, D], F32R)
    afdram = nc.dram_tensor("afdram", [P, 4, NG], F32R)   # aggr, feature-major

    with tile.TileContext(nc) as tc:
        with (
            tc.tile_pool(name="cst", bufs=1) as cst,
            tc.tile_pool(name="sb", bufs=2) as sb,
            tc.tile_pool(name="ps", bufs=2, space="PSUM") as ps,
        ):
            # ---------- constants ----------
            ident = cst.tile([P, P], F32R)
            nc.sync.dma_start(out=ident[:], in_=ident_t[:])
            iota_i = cst.tile([P, P], I32)
            nc.gpsimd.iota(iota_i[:], pattern=[[1, P]], base=0, channel_multiplier=0)
            iota_f = cst.tile([P, P], F32)
            nc.vector.tensor_copy(out=iota_f[:], in_=iota_i[:])
            eps_tile = cst.tile([P, 1], F32)
            nc.vector.memset(eps_tile[:], LN_EPS)
            ones_sb = cst.tile([1, P], F32R)
            nc.sync.dma_start(out=ones_sb[:], in_=ones_t[:])
            b1row = cst.tile([1, D], F32R)
            nc.sync.dma_start(out=b1row[:], in_=b1row_t[:])
            b_h1 = cst.tile([P, 4], F32)
            nc.sync.dma_start(out=b_h1[:], in_=b_h1_t[:])
            b_n1 = cst.tile([P, 4], F32)
            nc.sync.dma_start(out=b_n1[:], in_=b_n1_t[:])
            b_f1 = cst.tile([P, 4], F32)
            nc.sync.dma_start(out=b_f1[:], in_=b_f1_t[:])
            w_h1 = cst.tile([EDGE_DIM, D], F32R)
            nc.sync.dma_start(out=w_h1[:], in_=w_h1_t[:])
            if has_b2_e:
                b2e = cst.tile([1, D], F32R)
                nc.sync.dma_start(out=b2e[:], in_=b2e_t[:])
            if has_b2_g:
                b2g = cst.tile([1, D], F32R)
                nc.sync.dma_start(out=b2g[:], in_=b2g_t[:])
            if has_b2_n:
                b2n = cst.tile([1, D], F32R)
                nc.sync.dma_start(out=b2n[:], in_=b2n_t[:])
            if has_b2_f:
                b2f = cst.tile([1, 256], F32R)
                nc.sync.dma_start(out=b2f[:], in_=b2f_t[:])
            if has_deg:
                deg_sb = cst.tile([1, NG], F32R)
                nc.sync.dma_start(out=deg_sb[:], in_=deg_t[:])
                cdeg = cst.tile([1, D], F32R)
                nc.sync.dma_start(out=cdeg[:], in_=cdeg_t[:])
            if has_fin_aff:
                fing = cst.tile([P, OUT_DIM], F32)
                nc.sync.dma_start(out=fing[:], in_=fing_t[:])
                finb = cst.tile([P, OUT_DIM], F32)
                nc.sync.dma_start(out=finb[:], in_=finb_t[:])

            # ---------- phase A: A_g = Xg @ W1a + b1row -> agdram ----------
            w_ag = sb.tile([P, 4, D], F32R, tag="w4", bufs=3)
            nc.sync.dma_start(out=w_ag[:], in_=w_ag_t[:])
            w_am = sb.tile([P, 4, D], F32R, tag="w4", bufs=3)
            nc.sync.dma_start(out=w_am[:], in_=w_am_t[:])
            for nt in range(NB):
                xga = sb.tile([P, 4, P], F32R, tag="mch", bufs=3, name="xga")
                nc.sync.dma_start(out=xga[:], in_=xgT_t[:, :, nt * P:(nt + 1) * P])
                pag = ps.tile([P, D], F32, tag="mm", bufs=5, name="pag")
                for kt in range(4):
                    nc.tensor.matmul(out=pag[:], lhsT=xga[:, kt, :],
                                     rhs=w_ag[:, kt, :], start=(kt == 0),
                                     stop=(kt == 3 and not has_b1row))
                if has_b1row:
                    nc.tensor.matmul(out=pag[:], lhsT=ones_sb[0:1, :],
                                     rhs=b1row[0:1, :], start=False, stop=True)
                ago = sb.tile([P, D], F32R, tag="evac", bufs=4, name="ago")
                nc.scalar.copy(out=ago[:], in_=pag[:])
                nc.sync.dma_start(out=agdram[nt * P:(nt + 1) * P, :], in_=ago[:])
            # A_m = mesh @ W1b -> amdram
            for mc in range(MESH_PAD // P):
                mch = sb.tile([P, 4, P], F32R, tag="mch", bufs=3, name="mch")
                nc.sync.dma_start(out=mch[:], in_=meshT_t[:, :, mc * P:(mc + 1) * P])
                pam = ps.tile([P, D], F32, tag="mm", bufs=5, name="pam")
                for kt in range(4):
                    nc.tensor.matmul(out=pam[:], lhsT=mch[:, kt, :],
                                     rhs=w_am[:, kt, :], start=(kt == 0),
                                     stop=(kt == 3))
                amo = sb.tile([P, D], F32R, tag="evac", bufs=4, name="amo")
                nc.scalar.copy(out=amo[:], in_=pam[:])
                nc.sync.dma_start(out=amdram[mc * P:(mc + 1) * P, :], in_=amo[:])

            # ---------- phase B: edges ----------
            w_h2 = sb.tile([P, 4, D], F32R, tag="w4", bufs=3)
            nc.sync.dma_start(out=w_h2[:], in_=w_h2_t[:])
            w_ce = sb.tile([P, 4, D], F32R, tag="w4", bufs=3)
            nc.sync.dma_start(out=w_ce[:], in_=w_ce_t[:])
            w_g2 = sb.tile([P, 4, D], F32R, tag="w4", bufs=3)
            nc.sync.dma_start(out=w_g2[:], in_=w_g2_t[:])

            # Edge phase: 3-stage software pipeline over e-tiles so the PE
            # never waits for a tile's LN/assembly chain (runs on DVE/ACT).
            all_tiles = [(b, t0, p) for b in range(NB) for (t0, p) in tiles_per_blk]
            T = len(tiles_per_blk)
            ctxs = [dict() for _ in all_tiles]
            blk_msgs = {b: [] for b in range(NB)}
            pending_scatter = []

            def stage1(i):
                b, t0, p = all_tiles[i]
                cx = ctxs[i]
                if t0 == 0:
                    attrb = sb.tile([EDGE_DIM, c_blk], F32R, tag="attrb", bufs=2,
                                    name="attrb")
                    nc.sync.dma_start(out=attrb[:], in_=attrT_t[:, b * c_blk:(b + 1) * c_blk])
                    h1F = sb.tile([P, 4, c_blk], F32R, tag="h1F", bufs=2, name="h1F")
                    for m in range(4):
                        ph1 = ps.tile([P, c_blk], F32, tag="mm", bufs=5, name="ph1")
                        nc.tensor.matmul(out=ph1[:], lhsT=w_h1[:, m * P:(m + 1) * P],
                                         rhs=attrb[:], start=True, stop=True)
                        nc.scalar.activation(out=h1F[:, m, :], in_=ph1[:], func=AF.Relu,
                                             bias=b_h1[:, m:m + 1], scale=1.0)
                    ctxs[i]["h1F"] = h1F
                else:
                    ctxs[i]["h1F"] = ctxs[i - 1]["h1F"]
                h1F = cx["h1F"]
                # h2 = h1 @ W2 (+b2e), then LN -> e
                ph2 = ps.tile([P, D], F32, tag="mm", bufs=5, name="ph2")
                for kt in range(4):
                    nc.tensor.matmul(out=ph2[:p], lhsT=h1F[:, kt, t0:t0 + p],
                                     rhs=w_h2[:, kt, :], start=(kt == 0),
                                     stop=(kt == 3 and not has_b2_e))
                if has_b2_e:
                    nc.tensor.matmul(out=ph2[:p], lhsT=ones_sb[0:1, :p],
                                     rhs=b2e[0:1, :], start=False, stop=True)
                e_sb = sb.tile([P, D], F32R, tag="e_sb", bufs=3, name="e_sb")
                _ln_evac(nc, sb, ph2, e_sb[:p, :], p, eps_tile, D, "e")
                cx["e"] = e_sb

            def stage2(i):
                b, t0, p = all_tiles[i]
                cx = ctxs[i]
                e0 = b * c_blk + t0
                # gathers (DMA; consumed in stage3)
                recvl = sb.tile([P, 1], I32, tag="recvl", bufs=3, name="recvl")
                nc.sync.dma_start(out=recvl[:p], in_=recvl_t[e0:e0 + p, :])
                send = sb.tile([P, 1], I32, tag="send", bufs=3, name="send")
                nc.sync.dma_start(out=send[:p], in_=send_t[e0:e0 + p, :])
                ag = sb.tile([P, D], F32R, tag="ag", bufs=4, name="ag")
                nc.gpsimd.indirect_dma_start(
                    out=ag[:p], out_offset=None, in_=agdram[:],
                    in_offset=bass.IndirectOffsetOnAxis(ap=recvl[:p, :1], axis=0))
                am = sb.tile([P, D], F32R, tag="am", bufs=4, name="am")
                nc.gpsimd.indirect_dma_start(
                    out=am[:p], out_offset=None, in_=amdram[:],
                    in_offset=bass.IndirectOffsetOnAxis(ap=send[:p, :1], axis=0))
                cx["ag"], cx["am"] = ag, am
                recvb = sb.tile([P, 1], F32, tag="recvb", bufs=3, name="recvb")
                nc.sync.dma_start(out=recvb[:p], in_=recvb_t[e0:e0 + p, :])
                cx["recvb"] = recvb
                # eF = e.T ; Ce = e @ W1c
                e_sb = cx.pop("e")
                ptr = ps.tile([P, 4, P], F32R, tag="tr", bufs=3, name="ptr")
                for j in range(4):
                    nc.tensor.transpose(out=ptr[:, j, :p],
                                        in_=e_sb[:p, j * P:(j + 1) * P],
                                        identity=ident[:p, :p])
                eF = sb.tile([P, 4, P], F32R, tag="eF", bufs=3, name="eF")
                nc.vector.tensor_copy(out=eF[:, :, :p], in_=ptr[:, :, :p])
                pce = ps.tile([P, D], F32, tag="mm", bufs=5, name="pce")
                for kt in range(4):
                    nc.tensor.matmul(out=pce[:p], lhsT=eF[:, kt, :p],
                                     rhs=w_ce[:, kt, :], start=(kt == 0), stop=False)
                ag, am = cx.pop("ag"), cx.pop("am")
                u = sb.tile([P, D], F32R, tag="u", bufs=2, name="u")
                nc.vector.tensor_tensor(out=u[:p], in0=ag[:p], in1=am[:p],
                                        op=OP.add)
                nc.tensor.matmul(out=pce[:p], lhsT=ident[:p, :p], rhs=u[:p],
                                 start=False, stop=True)
                cx["pce"] = pce

            def stage3(i):
                b, t0, p = all_tiles[i]
                cx = ctxs[i]
                pce = cx.pop("pce")
                # m1 = relu(Ce + Ag + Am)  (adds already accumulated on PE)
                m1 = sb.tile([P, D], F32R, tag="m1", bufs=2, name="m1")
                nc.scalar.activation(out=m1[:p, :], in_=pce[:p], func=AF.Relu)
                ptr2 = ps.tile([P, 4, P], F32R, tag="tr", bufs=3, name="ptr2")
                for j in range(4):
                    nc.tensor.transpose(out=ptr2[:, j, :p],
                                        in_=m1[:p, j * P:(j + 1) * P],
                                        identity=ident[:p, :p])
                m1F = sb.tile([P, 4, P], F32R, tag="m1F", bufs=3, name="m1F")
                nc.vector.tensor_copy(out=m1F[:, :, :p], in_=ptr2[:, :, :p])
                pg2 = ps.tile([P, D], F32, tag="mm", bufs=5, name="pg2")
                for kt in range(4):
                    nc.tensor.matmul(out=pg2[:p], lhsT=m1F[:, kt, :p],
                                     rhs=w_g2[:, kt, :], start=(kt == 0),
                                     stop=(kt == 3 and not has_b2_g))
                if has_b2_g:
                    nc.tensor.matmul(out=pg2[:p], lhsT=ones_sb[0:1, :p],
                                     rhs=b2g[0:1, :], start=False, stop=True)
                msg = sb.tile([P, D], F32R, tag="msg", bufs=6, name="msg")
                _ln_evac(nc, sb, pg2, msg[:p, :], p, eps_tile, D, "g")
                recvb = cx.pop("recvb")
                oh = sb.tile([P, P], F32R, tag="oh", bufs=6, name="oh")
                nc.vector.tensor_scalar(out=oh[:p, :], in0=iota_f[:p, :],
                                        scalar1=recvb[:p, 0:1], scalar2=None,
                                        op0=OP.is_equal)
                blk_msgs[b].append((msg, oh, p))
                if len(blk_msgs[b]) == T:
                    pending_scatter.append(b)

            def emit_scatter(b):
                pblk = ps.tile([P, D], F32, tag="tr", bufs=3, name="pblk")
                for k, (msg, oh, p) in enumerate(blk_msgs[b]):
                    nc.tensor.matmul(out=pblk[:], lhsT=oh[:p, :], rhs=msg[:p, :],
                                     start=(k == 0), stop=(k == T - 1))
                ab = sb.tile([P, D], F32R, tag="ab", bufs=2, name="ab")
                nc.scalar.copy(out=ab[:], in_=pblk[:])
                ptr3 = ps.tile([P, 4, P], F32R, tag="tr", bufs=3, name="ptr3")
                for j in range(4):
                    nc.tensor.transpose(out=ptr3[:, j, :],
                                        in_=ab[:, j * P:(j + 1) * P],
                                        identity=ident[:])
                abF = sb.tile([P, 4, P], F32R, tag="abF", bufs=2, name="abF")
                nc.vector.tensor_copy(out=abF[:], in_=ptr3[:])
                nc.sync.dma_start(out=afdram[:, :, b * P:(b + 1) * P], in_=abF[:])

            NT = len(all_tiles)
            for i in range(NT + 2):
                if i < NT:
                    stage1(i)
                while pending_scatter:
                    emit_scatter(pending_scatter.pop(0))
                if 0 <= i - 1 < NT:
                    stage2(i - 1)
                if 0 <= i - 2 < NT:
                    stage3(i - 2)
            while pending_scatter:
                emit_scatter(pending_scatter.pop(0))

            # ---------- phase C: node MLPs ----------
            w_n1a = sb.tile([P, 4, D], F32R, tag="w4", bufs=3)
            nc.sync.dma_start(out=w_n1a[:], in_=w_n1_t[:, 0:4, :])
            w_n1b = sb.tile([P, 4, D], F32R, tag="w4", bufs=3)
            nc.sync.dma_start(out=w_n1b[:], in_=w_n1_t[:, 4:8, :])
            w_n2 = sb.tile([P, 4, D], F32R, tag="w4", bufs=3)
            nc.sync.dma_start(out=w_n2[:], in_=w_n2_t[:])
            w_f1a = sb.tile([P, 4, D], F32R, tag="w4b", bufs=3)
            nc.sync.dma_start(out=w_f1a[:], in_=w_f1_t[:, 0:4, :])
            w_f1b = sb.tile([P, 4, D], F32R, tag="w4b", bufs=3)
            nc.sync.dma_start(out=w_f1b[:], in_=w_f1_t[:, 4:8, :])
            w_f2 = sb.tile([P, 4, 256], F32R, tag="w4b", bufs=3)
            nc.sync.dma_start(out=w_f2[:], in_=w_f2_t[:])

            for c in range(NG // CH):
                c0 = c * CH
                xgc = sb.tile([P, 4, CH], F32R, tag="xgc", bufs=2, name="xgc")
                nc.sync.dma_start(out=xgc[:], in_=xgT_t[:, :, c0:c0 + CH])
                afc = sb.tile([P, 4, CH], F32R, tag="afc", bufs=2, name="afc")
                nc.sync.dma_start(out=afc[:], in_=afdram[:, :, c0:c0 + CH])
                # n1F = relu(W_n1.T @ [Xg; aggr])  F-major [512, CH]
                n1F = sb.tile([P, 4, CH], F32R, tag="n1F", bufs=2, name="n1F")
                for m in range(4):
                    pn1 = ps.tile([P, CH], F32, tag="mm", bufs=5, name="pn1")
                    for kt in range(8):
                        lhs = (w_n1a[:, kt, m * P:(m + 1) * P] if kt < 4
                               else w_n1b[:, kt - 4, m * P:(m + 1) * P])
                        rhs = xgc[:, kt, :] if kt < 4 else afc[:, kt - 4, :]
                        nc.tensor.matmul(out=pn1[:], lhsT=lhs, rhs=rhs,
                                         start=(kt == 0),
                                         stop=(kt == 7 and not has_deg))
                    if has_deg:
                        nc.tensor.matmul(out=pn1[:], lhsT=cdeg[0:1, m * P:(m + 1) * P],
                                         rhs=deg_sb[0:1, c0:c0 + CH],
                                         start=False, stop=True)
                    nc.scalar.activation(out=n1F[:, m, :], in_=pn1[:], func=AF.Relu,
                                         bias=b_n1[:, m:m + 1], scale=1.0)
                # per node-tile: n2, LN -> ygn, transpose
                ygnF = sb.tile([P, 4, CH], F32R, tag="ygnF", bufs=2, name="ygnF")
                for ntl in range(CH // P):
                    s0 = ntl * P
                    pn2 = ps.tile([P, D], F32, tag="mm", bufs=5, name="pn2")
                    for kt in range(4):
                        nc.tensor.matmul(out=pn2[:], lhsT=n1F[:, kt, s0:s0 + P],
                                         rhs=w_n2[:, kt, :], start=(kt == 0),
                                         stop=(kt == 3 and not has_b2_n))
                    if has_b2_n:
                        nc.tensor.matmul(out=pn2[:], lhsT=ones_sb[0:1, :P],
                                         rhs=b2n[0:1, :], start=False, stop=True)
                    ygn = sb.tile([P, D], F32R, tag="ygn", bufs=2, name="ygn")
                    _ln_evac(nc, sb, pn2, ygn[:, :], P, eps_tile, D, "n")
                    ptr4 = ps.tile([P, 4, P], F32R, tag="tr", bufs=3, name="ptr4")
                    for j in range(4):
                        nc.tensor.transpose(out=ptr4[:, j, :],
                                            in_=ygn[:, j * P:(j + 1) * P],
                                            identity=ident[:])
                    nc.vector.tensor_copy(out=ygnF[:, :, s0:s0 + P], in_=ptr4[:])
                # f1F = relu(W_f1.T @ [Xg; ygn])
                f1F = sb.tile([P, 4, CH], F32R, tag="f1F", bufs=2, name="f1F")
                for m in range(4):
                    pf1 = ps.tile([P, CH], F32, tag="mm", bufs=5, name="pf1")
                    for kt in range(8):
                        lhs = (w_f1a[:, kt, m * P:(m + 1) * P] if kt < 4
                               else w_f1b[:, kt - 4, m * P:(m + 1) * P])
                        rhs = xgc[:, kt, :] if kt < 4 else ygnF[:, kt - 4, :]
                        nc.tensor.matmul(out=pf1[:], lhsT=lhs, rhs=rhs,
                                         start=(kt == 0), stop=(kt == 7))
                    nc.scalar.activation(out=f1F[:, m, :], in_=pf1[:], func=AF.Relu,
                                         bias=b_f1[:, m:m + 1], scale=1.0)
                # f2 + final LN (+affine) -> out
                for ntl in range(CH // P):
                    s0 = ntl * P
                    pf2 = ps.tile([P, 256], F32, tag="mm", bufs=5, name="pf2")
                    for kt in range(4):
                        nc.tensor.matmul(out=pf2[:], lhsT=f1F[:, kt, s0:s0 + P],
                                         rhs=w_f2[:, kt, :], start=(kt == 0),
                                         stop=(kt == 3 and not has_b2_f))
                    if has_b2_f:
                        nc.tensor.matmul(out=pf2[:], lhsT=ones_sb[0:1, :P],
                                         rhs=b2f[0:1, :], start=False, stop=True)
                    o_sb = sb.tile([P, OUT_DIM], F32, tag="o_sb", bufs=3, name="o_sb")
                    _ln_evac(nc, sb, pf2, o_sb[:, :], P, eps_tile, OUT_DIM, "f")
                    if has_fin_aff:
                        nc.vector.tensor_tensor(out=o_sb[:], in0=o_sb[:],
                                                in1=fing[:], op=OP.mult)
                        nc.vector.tensor_tensor(out=o_sb[:], in0=o_sb[:],
                                                in1=finb[:], op=OP.add)
                    nc.sync.dma_start(out=out_t[c0 + s0:c0 + s0 + P, :], in_=o_sb[:])

    nc.finalize()
    _fix_multi_waits(nc)
    return nc


_cache = {}


def _get_module(c_blk, flags, mm_dt):
    key = (c_blk, flags, str(mm_dt))
    if key not in _cache:
        _cache[key] = _build(c_blk, flags, mm_dt)
    return _cache[key]


def kernel(**inputs):
    _install_ntff_shim()
    f32 = np.float32

    grid = np.asarray(inputs["input_grid_nodes"], f32)
    mesh = np.asarray(inputs["input_mesh_nodes"], f32)
    attr = np.asarray(inputs["input_edge_attr"], f32)
    ei = np.asarray(inputs["edge_index"])
    senders, receivers = ei[0].astype(np.int64), ei[1].astype(np.int64)

    g = {k: np.asarray(v, f32) for k, v in inputs.items()
         if k not in ("input_grid_nodes", "input_mesh_nodes", "input_edge_attr",
                      "edge_index")}

    # ---- fold LN affines / biases into weights (host) ----
    w_h1 = g["emlp_w1"]                        # [4, 512] lhsT
    b_h1v = g["emlp_b1"]                       # per-feature -> evac bias (F-major)
    w_h2 = g["emlp_w2"]                        # [512, 512]
    b2e = g["emlp_b2"]
    ge_w1 = g["ge_w1"]                         # [1536, 512]
    w_ag_m = ge_w1[0:D]
    w_am_m = ge_w1[D:2 * D]
    w_ce_m = g["emlp_g"][:, None] * ge_w1[2 * D:3 * D]
    b1row_v = g["ge_b1"] + g["emlp_beta"] @ ge_w1[2 * D:3 * D]
    w_g2 = g["ge_w2"]
    b2g = g["ge_b2"]
    w_n1_m = np.concatenate([g["gn_w1"][0:D], g["ge_g"][:, None] * g["gn_w1"][D:2 * D]], 0)
    # ge_beta flows through the scatter as deg[n] * (ge_beta @ gn_w1b): rank-1 term
    cdeg_v = g["ge_beta"] @ g["gn_w1"][D:2 * D]
    b_n1v = g["gn_b1"]
    w_n2 = g["gn_w2"]
    b2n = g["gn_b2"]
    w_f1_m = np.concatenate([g["fin_w1"], g["gn_g"][:, None] * g["fin_w1"]], 0)
    b_f1v = g["fin_b1"] + g["gn_beta"] @ g["fin_w1"]
    w_f2 = np.zeros((D, 256), f32)
    w_f2[:, :OUT_DIM] = g["fin_w2"]
    b2f = np.zeros((1, 256), f32)
    b2f[0, :OUT_DIM] = g["fin_b2"]
    fin_g, fin_beta = g["fin_g"], g["fin_beta"]

    flags = (
        bool(np.any(b2e)), bool(np.any(b2g)), bool(np.any(b2n)),
        bool(np.any(g["fin_b2"])), bool(np.any(g["ge_beta"])),
        bool(np.any(fin_beta) or not np.all(fin_g == 1.0)),
        bool(np.any(b1row_v)),
    )

    # ---- shard edges by receiver, per-block padding ----
    perm = np.argsort(receivers, kind="stable")
    r_s, s_s = receivers[perm], senders[perm]
    attr_s = attr[perm]
    blk = (r_s // P).astype(np.int64)                    # global block id 0..127
    bc = np.bincount(blk, minlength=NC_ * NB)
    c_blk = max(P, int(-(-bc.max() // 64)) * 64)
    e_pad = NB * c_blk
    tiles_per_blk = [(t * P, min(P, c_blk - t * P)) for t in range((c_blk + P - 1) // P)]

    # destination slot per edge: block_base + rank within block
    blk_starts = np.zeros(NC_ * NB + 1, np.int64)
    np.cumsum(bc, out=blk_starts[1:])
    rank = np.arange(len(r_s)) - blk_starts[blk]
    core = blk // NB
    slot = (blk % NB) * c_blk + rank                     # slot within core

    send_a = np.zeros((NC_, e_pad, 1), np.int32)
    recvl_a = np.zeros((NC_, e_pad, 1), np.int32)
    recvb_a = np.full((NC_, e_pad, 1), -1.0, f32)
    attr_a = np.zeros((NC_, EDGE_DIM, e_pad), f32)
    send_a[core, slot, 0] = s_s
    recvl_a[core, slot, 0] = r_s - core * NG
    recvb_a[core, slot, 0] = (r_s % P).astype(f32)
    attr_a[core, :, slot] = attr_s

    meshT = np.zeros((D, MESH_PAD), f32)
    meshT[:, :N_MESH] = mesh.T
    deg = np.bincount(receivers, minlength=N_GRID).astype(f32)

    def pack_rhs(w):                                     # [K, N] -> [128, K/128, N]
        return np.ascontiguousarray(w.reshape(-1, P, w.shape[1]).transpose(1, 0, 2))

    def pack_bias(v):                                    # [512] -> [128, 4]
        return np.ascontiguousarray(v.reshape(4, P).T)

    rep = {
        "meshT": np.ascontiguousarray(
            meshT.reshape(4, P, MESH_PAD).transpose(1, 0, 2)),
        "ident": np.eye(P, dtype=f32),
        "ones": np.ones((1, P), f32),
        "w_h1": np.ascontiguousarray(w_h1),
        "b_h1": pack_bias(b_h1v),
        "w_h2": pack_rhs(w_h2),
        "w_ag": pack_rhs(w_ag_m),
        "w_am": pack_rhs(w_am_m),
        "w_ce": pack_rhs(w_ce_m),
        "w_g2": pack_rhs(w_g2),
        "b1row": b1row_v.reshape(1, D).astype(f32),
        "w_n1": pack_rhs(w_n1_m),
        "b_n1": pack_bias(b_n1v),
        "w_n2": pack_rhs(w_n2),
        "w_f1": pack_rhs(w_f1_m),
        "b_f1": pack_bias(b_f1v),
        "w_f2": pack_rhs(w_f2),
        "b2e": b2e.reshape(1, D), "b2g": b2g.reshape(1, D),
        "b2n": b2n.reshape(1, D), "b2f": b2f,
        "cdeg": cdeg_v.reshape(1, D),
        "fing": np.broadcast_to(fin_g, (P, OUT_DIM)).copy(),
        "finb": np.broadcast_to(fin_beta, (P, OUT_DIM)).copy(),
    }

    in_maps = []
    for c in range(NC_):
        xg = grid[c * NG:(c + 1) * NG]
        m = dict(rep)
        m["xgT"] = np.ascontiguousarray(
            xg.T.reshape(4, P, NG).transpose(1, 0, 2))
        m["attrT"] = attr_a[c]
        m["send"] = send_a[c]
        m["recvl"] = recvl_a[c]
        m["recvb"] = recvb_a[c]
        m["deg"] = deg[c * NG:(c + 1) * NG].reshape(1, NG)
        in_maps.append(m)

    use_bf16 = os.environ.get("KERNEL_MM_DT", "f32r") == "bf16"
    mm_dt = mybir.dt.bfloat16 if use_bf16 else mybir.dt.float32r
    if use_bf16:
        import ml_dtypes
        bf16 = ml_dtypes.bfloat16
        mm_keys = ["xgT", "attrT", "meshT", "ident", "ones", "w_h1", "w_h2",
                   "w_ag", "w_am", "w_ce", "w_g2", "b1row", "w_n1", "w_n2",
                   "w_f1", "w_f2", "b2e", "b2g", "b2n", "b2f", "deg", "cdeg"]
        for m in in_maps:
            for k in mm_keys:
                if k in m:
                    m[k] = np.asarray(m[k]).astype(bf16)
    nc = _get_module(c_blk, flags, mm_dt)
    trace = bool(int(os.environ.get("KERNEL_TRACE", "0")))
    res = run_bass_kernel_spmd(nc, in_maps, core_ids=list(range(NC_)), trace=trace)
    if trace:
        kernel.last_exec_time_ns = res.exec_time_ns
        kernel.last_results = res
    return np.concatenate([res.results[c]["out"] for c in range(NC_)], axis=0)


# revision 21
# speedup vs baseline: 1.0803x; 1.0803x over previous
"""GNN decoder (grid<-mesh message passing) as a Bass/Tile kernel on 8 TRN2 NeuronCores.

Strategy (graph/data parallel per the sharding hint):
  - Grid nodes sharded 2048/core; each edge lives on its receiver's core, sorted by
    receiver and padded per 128-node block so scatter-add is a local one-hot matmul.
  - Mesh table + all weights replicated.
  - Edge message MLP first layer decomposed: cat(x_i,x_j,e) @ W1 =
    (grid@W1a)[recv] + (mesh@W1b)[send] + e@W1c  -> per-node precompute + row gathers.
  - LayerNorm affine/g/beta and all biases folded into weights / evac bias on host;
    LN itself = bn_stats on PSUM + normalize fused into the PSUM->SBUF evacuation.
  - All matmuls in float32r (full PE rate, ~1e-4 rel err).
"""
import os
import sys
import types

import numpy as np

_TRN_REPO = "/opt/trn_rl_repo"
if _TRN_REPO not in sys.path:
    sys.path.append(_TRN_REPO)

import concourse.bass as bass
import concourse.tile as tile
from concourse import mybir
from concourse.bass_utils import run_bass_kernel_spmd

F32 = mybir.dt.float32
F32R = mybir.dt.float32r
I32 = mybir.dt.int32
AF = mybir.ActivationFunctionType
OP = mybir.AluOpType

N_MESH, N_GRID, N_EDGE = 2562, 16384, 49152
D, EDGE_DIM, OUT_DIM = 512, 4, 128
LN_EPS = 1e-5
NC_ = 8                      # cores
NG = N_GRID // NC_           # 2048 grid nodes per core
NB = NG // 128               # 16 blocks per core
MESH_PAD = 2688              # 21 * 128
P = 128


def _fix_multi_waits(nc, max_waits=1):
    """walrus setupSyncWait rejects >~2 sem waits per instruction; hoist extras
    onto preceding same-engine no-op carriers (cheap, unlike a GpSimd DRAIN)."""
    for f in nc.m.functions:
        for b in f.blocks:
            new_insts = []
            for inst in b.instructions:
                si = inst.sync_info
                if si is not None and len(si.on_wait) > max_waits:
                    waits = list(si.on_wait)
                    extra, keep = waits[:-max_waits], waits[-max_waits:]
                    for i in range(0, len(extra), max_waits):
                        d = mybir.InstEventSemaphore(
                            name=nc.get_next_instruction_name(),
                            ins=[], outs=[])
                        d.engine = inst.engine
                        d.sync_info = mybir.SyncInfo(
                            on_wait=extra[i:i + max_waits], on_update=[])
                        nc.register_instruction(d, overwrite=True)
                        new_insts.append(d)
                    inst.sync_info = mybir.SyncInfo(
                        on_wait=keep, on_update=list(si.on_update))
                new_insts.append(inst)
            b.instructions = new_insts


def _install_ntff_shim():
    """This image's antenv lacks axon_hooks; shim it so trace=True works."""
    if 'antenv.axon_hooks' in sys.modules:
        return
    mod = types.ModuleType('antenv.axon_hooks')
    _hook = [None]
    mod.set_axon_ntff_profile_hook = lambda h: _hook.__setitem__(0, h)
    mod.get_axon_ntff_profile_hook = lambda: _hook[0]
    sys.modules['antenv.axon_hooks'] = mod
    try:
        import antenv
        antenv.axon_hooks = mod
        from trn_agent_boot.trn_boot import _ntff_profile_via_ctypes
        mod.set_axon_ntff_profile_hook(
            _ntff_profile_via_ctypes('/opt/axon/libaxon_pjrt.so'))
    except Exception:
        pass


def _ln_evac(nc, sb, ps_in, out_ap, p, eps_tile, free, tag):
    """out = LN(ps_in) (no affine): bn_stats on PSUM, normalize during evacuation."""
    stats = sb.tile([P, 6], F32, tag=f"st_{tag}", name=f"st_{tag}")
    nc.vector.bn_stats(out=stats[:p, :], in_=ps_in[:p, :free])
    mv = sb.tile([P, 2], F32, tag=f"mv_{tag}", name=f"mv_{tag}")
    nc.vector.bn_aggr(out=mv[:p, :], in_=stats[:p, :])
    rstd = sb.tile([P, 1], F32, tag=f"rs_{tag}", name=f"rs_{tag}")
    nc.scalar.activation(out=rstd[:p], in_=mv[:p, 1:2], func=AF.Sqrt,
                         bias=eps_tile[:p], scale=1.0)
    nc.vector.reciprocal(out=rstd[:p], in_=rstd[:p])
    negmr = sb.tile([P, 1], F32, tag=f"nm_{tag}", name=f"nm_{tag}")
    nc.vector.tensor_scalar(out=negmr[:p], in0=mv[:p, 0:1], scalar1=rstd[:p],
                            scalar2=-1.0, op0=OP.mult, op1=OP.mult)
    nc.scalar.activation(out=out_ap, in_=ps_in[:p, :free], func=AF.Identity,
                         bias=negmr[:p], scale=rstd[:p])


def _build(c_blk, flags, mm_dt=None):
    """Build the SPMD Bass module for per-block edge capacity c_blk."""
    F32R = mm_dt if mm_dt is not None else mybir.dt.float32r
    (has_b2_e, has_b2_g, has_b2_n, has_b2_f, has_deg, has_fin_aff, has_b1row) = flags
    e_pad = NB * c_blk
    tiles_per_blk = [(t * P, min(P, c_blk - t * P))
                     for t in range((c_blk + P - 1) // P)]
    CH = 256                      # node-chunk width in phase C

    nc = bass.Bass()
    # ---- per-core inputs ----
    xgT_t = nc.dram_tensor("xgT", [P, 4, NG], F32R, kind="ExternalInput")
    attrT_t = nc.dram_tensor("attrT", [EDGE_DIM, e_pad], F32R, kind="ExternalInput")
    send_t = nc.dram_tensor("send", [e_pad, 1], I32, kind="ExternalInput")
    recvl_t = nc.dram_tensor("recvl", [e_pad, 1], I32, kind="ExternalInput")
    recvb_t = nc.dram_tensor("recvb", [e_pad, 1], F32, kind="ExternalInput")
    # ---- replicated inputs ----
    meshT_t = nc.dram_tensor("meshT", [P, 4, MESH_PAD], F32R, kind="ExternalInput")
    ident_t = nc.dram_tensor("ident", [P, P], F32R, kind="ExternalInput")
    ones_t = nc.dram_tensor("ones", [1, P], F32R, kind="ExternalInput")
    w_h1_t = nc.dram_tensor("w_h1", [EDGE_DIM, D], F32R, kind="ExternalInput")
    b_h1_t = nc.dram_tensor("b_h1", [P, 4], F32, kind="ExternalInput")
    w_h2_t = nc.dram_tensor("w_h2", [P, 4, D], F32R, kind="ExternalInput")
    w_ag_t = nc.dram_tensor("w_ag", [P, 4, D], F32R, kind="ExternalInput")
    w_am_t = nc.dram_tensor("w_am", [P, 4, D], F32R, kind="ExternalInput")
    w_ce_t = nc.dram_tensor("w_ce", [P, 4, D], F32R, kind="ExternalInput")
    w_g2_t = nc.dram_tensor("w_g2", [P, 4, D], F32R, kind="ExternalInput")
    b1row_t = nc.dram_tensor("b1row", [1, D], F32R, kind="ExternalInput")
    w_n1_t = nc.dram_tensor("w_n1", [P, 8, D], F32R, kind="ExternalInput")
    b_n1_t = nc.dram_tensor("b_n1", [P, 4], F32, kind="ExternalInput")
    w_n2_t = nc.dram_tensor("w_n2", [P, 4, D], F32R, kind="ExternalInput")
    w_f1_t = nc.dram_tensor("w_f1", [P, 8, D], F32R, kind="ExternalInput")
    b_f1_t = nc.dram_tensor("b_f1", [P, 4], F32, kind="ExternalInput")
    w_f2_t = nc.dram_tensor("w_f2", [P, 4, 256], F32R, kind="ExternalInput")
    # optional generality inputs (always declared; tiny)
    b2e_t = nc.dram_tensor("b2e", [1, D], F32R, kind="ExternalInput")
    b2g_t = nc.dram_tensor("b2g", [1, D], F32R, kind="ExternalInput")
    b2n_t = nc.dram_tensor("b2n", [1, D], F32R, kind="ExternalInput")
    b2f_t = nc.dram_tensor("b2f", [1, 256], F32R, kind="ExternalInput")
    deg_t = nc.dram_tensor("deg", [1, NG], F32R, kind="ExternalInput")
    cdeg_t = nc.dram_tensor("cdeg", [1, D], F32R, kind="ExternalInput")
    fing_t = nc.dram_tensor("fing", [P, OUT_DIM], F32, kind="ExternalInput")
    finb_t = nc.dram_tensor("finb", [P, OUT_DIM], F32, kind="ExternalInput")

    out_t = nc.dram_tensor("out", [NG, OUT_DIM], F32, kind="ExternalOutput")
    agdram = nc.dram_tensor("agdram", [NG, D], F32R)
    amdram = nc.dram_tensor("amdram", [MESH_PAD, D], F32R)
    afdram = nc.dram_tensor("afdram", [P, 4, NG], F32R)   # aggr, feature-major

    with tile.TileContext(nc) as tc:
        with (
            tc.tile_pool(name="cst", bufs=1) as cst,
            tc.tile_pool(name="sb", bufs=2) as sb,
            tc.tile_pool(name="ps", bufs=2, space="PSUM") as ps,
        ):
            # ---------- constants ----------
            ident = cst.tile([P, P], F32R)
            nc.sync.dma_start(out=ident[:], in_=ident_t[:])
            iota_i = cst.tile([P, P], I32)
            nc.gpsimd.iota(iota_i[:], pattern=[[1, P]], base=0, channel_multiplier=0)
            iota_f = cst.tile([P, P], F32)
            nc.vector.tensor_copy(out=iota_f[:], in_=iota_i[:])
            eps_tile = cst.tile([P, 1], F32)
            nc.vector.memset(eps_tile[:], LN_EPS)
            ones_sb = cst.tile([1, P], F32R)
            nc.sync.dma_start(out=ones_sb[:], in_=ones_t[:])
            b1row = cst.tile([1, D], F32R)
            nc.sync.dma_start(out=b1row[:], in_=b1row_t[:])
            b_h1 = cst.tile([P, 4], F32)
            nc.sync.dma_start(out=b_h1[:], in_=b_h1_t[:])
            b_n1 = cst.tile([P, 4], F32)
            nc.sync.dma_start(out=b_n1[:], in_=b_n1_t[:])
            b_f1 = cst.tile([P, 4], F32)
            nc.sync.dma_start(out=b_f1[:], in_=b_f1_t[:])
            w_h1 = cst.tile([EDGE_DIM, D], F32R)
            nc.sync.dma_start(out=w_h1[:], in_=w_h1_t[:])
            if has_b2_e:
                b2e = cst.tile([1, D], F32R)
                nc.sync.dma_start(out=b2e[:], in_=b2e_t[:])
            if has_b2_g:
                b2g = cst.tile([1, D], F32R)
                nc.sync.dma_start(out=b2g[:], in_=b2g_t[:])
            if has_b2_n:
                b2n = cst.tile([1, D], F32R)
                nc.sync.dma_start(out=b2n[:], in_=b2n_t[:])
            if has_b2_f:
                b2f = cst.tile([1, 256], F32R)
                nc.sync.dma_start(out=b2f[:], in_=b2f_t[:])
            if has_deg:
                deg_sb = cst.tile([1, NG], F32R)
                nc.sync.dma_start(out=deg_sb[:], in_=deg_t[:])
                cdeg = cst.tile([1, D], F32R)
                nc.sync.dma_start(out=cdeg[:], in_=cdeg_t[:])
            if has_fin_aff:
                fing = cst.tile([P, OUT_DIM], F32)
                nc.sync.dma_start(out=fing[:], in_=fing_t[:])
                finb = cst.tile([P, OUT_DIM], F32)
                nc.sync.dma_start(out=finb[:], in_=finb_t[:])

            # ---------- phase A: A_g = Xg @ W1a + b1row -> agdram ----------
            w_ag = sb.tile([P, 4, D], F32R, tag="w4", bufs=3)
            nc.sync.dma_start(out=w_ag[:], in_=w_ag_t[:])
            w_am = sb.tile([P, 4, D], F32R, tag="w4", bufs=3)
            nc.sync.dma_start(out=w_am[:], in_=w_am_t[:])
            for nt in range(NB):
                xga = sb.tile([P, 4, P], F32R, tag="mch", bufs=3, name="xga")
                nc.sync.dma_start(out=xga[:], in_=xgT_t[:, :, nt * P:(nt + 1) * P])
                pag = ps.tile([P, D], F32, tag="mm", bufs=5, name="pag")
                for kt in range(4):
                    nc.tensor.matmul(out=pag[:], lhsT=xga[:, kt, :],
                                     rhs=w_ag[:, kt, :], start=(kt == 0),
                                     stop=(kt == 3 and not has_b1row))
                if has_b1row:
                    nc.tensor.matmul(out=pag[:], lhsT=ones_sb[0:1, :],
                                     rhs=b1row[0:1, :], start=False, stop=True)
                ago = sb.tile([P, D], F32R, tag="evac", bufs=4, name="ago")
                nc.scalar.copy(out=ago[:], in_=pag[:])
                nc.sync.dma_start(out=agdram[nt * P:(nt + 1) * P, :], in_=ago[:])
            # A_m = mesh @ W1b -> amdram
            for mc in range(MESH_PAD // P):
                mch = sb.tile([P, 4, P], F32R, tag="mch", bufs=3, name="mch")
                nc.sync.dma_start(out=mch[:], in_=meshT_t[:, :, mc * P:(mc + 1) * P])
                pam = ps.tile([P, D], F32, tag="mm", bufs=5, name="pam")
                for kt in range(4):
                    nc.tensor.matmul(out=pam[:], lhsT=mch[:, kt, :],
                                     rhs=w_am[:, kt, :], start=(kt == 0),
                                     stop=(kt == 3))
                amo = sb.tile([P, D], F32R, tag="evac", bufs=4, name="amo")
                nc.scalar.copy(out=amo[:], in_=pam[:])
                nc.sync.dma_start(out=amdram[mc * P:(mc + 1) * P, :], in_=amo[:])

            # ---------- phase B: edges ----------
            w_h2 = sb.tile([P, 4, D], F32R, tag="w4", bufs=3)
            nc.sync.dma_start(out=w_h2[:], in_=w_h2_t[:])
            w_ce = sb.tile([P, 4, D], F32R, tag="w4", bufs=3)
            nc.sync.dma_start(out=w_ce[:], in_=w_ce_t[:])
            w_g2 = sb.tile([P, 4, D], F32R, tag="w4", bufs=3)
            nc.sync.dma_start(out=w_g2[:], in_=w_g2_t[:])

            # Edge phase: 3-stage software pipeline over e-tiles so the PE
            # never waits for a tile's LN/assembly chain (runs on DVE/ACT).
            all_tiles = [(b, t0, p) for b in range(NB) for (t0, p) in tiles_per_blk]
            T = len(tiles_per_blk)
            ctxs = [dict() for _ in all_tiles]
            blk_msgs = {b: [] for b in range(NB)}
            pending_scatter = []

            def stage1(i):
                b, t0, p = all_tiles[i]
                cx = ctxs[i]
                if t0 == 0:
                    attrb = sb.tile([EDGE_DIM, c_blk], F32R, tag="attrb", bufs=2,
                                    name="attrb")
                    nc.sync.dma_start(out=attrb[:], in_=attrT_t[:, b * c_blk:(b + 1) * c_blk])
                    h1F = sb.tile([P, 4, c_blk], F32R, tag="h1F", bufs=2, name="h1F")
                    for m in range(4):
                        ph1 = ps.tile([P, c_blk], F32, tag="mm", bufs=5, name="ph1")
                        nc.tensor.matmul(out=ph1[:], lhsT=w_h1[:, m * P:(m + 1) * P],
                                         rhs=attrb[:], start=True, stop=True)
                        nc.scalar.activation(out=h1F[:, m, :], in_=ph1[:], func=AF.Relu,
                                             bias=b_h1[:, m:m + 1], scale=1.0)
                    ctxs[i]["h1F"] = h1F
                else:
                    ctxs[i]["h1F"] = ctxs[i - 1]["h1F"]
                h1F = cx["h1F"]
                # h2 = h1 @ W2 (+b2e), then LN -> e
                ph2 = ps.tile([P, D], F32, tag="mm", bufs=5, name="ph2")
                for kt in range(4):
                    nc.tensor.matmul(out=ph2[:p], lhsT=h1F[:, kt, t0:t0 + p],
                                     rhs=w_h2[:, kt, :], start=(kt == 0),
                                     stop=(kt == 3 and not has_b2_e))
                if has_b2_e:
                    nc.tensor.matmul(out=ph2[:p], lhsT=ones_sb[0:1, :p],
                                     rhs=b2e[0:1, :], start=False, stop=True)
                e_sb = sb.tile([P, D], F32R, tag="e_sb", bufs=3, name="e_sb")
                _ln_evac(nc, sb, ph2, e_sb[:p, :], p, eps_tile, D, "e")
                cx["e"] = e_sb

            def stage2(i):
                b, t0, p = all_tiles[i]
                cx = ctxs[i]
                e0 = b * c_blk + t0
                # gathers (DMA; consumed in stage3)
                recvl = sb.tile([P, 1], I32, tag="recvl", bufs=3, name="recvl")
                nc.sync.dma_start(out=recvl[:p], in_=recvl_t[e0:e0 + p, :])
                send = sb.tile([P, 1], I32, tag="send", bufs=3, name="send")
                nc.sync.dma_start(out=send[:p], in_=send_t[e0:e0 + p, :])
                ag = sb.tile([P, D], F32R, tag="ag", bufs=4, name="ag")
                nc.gpsimd.indirect_dma_start(
                    out=ag[:p], out_offset=None, in_=agdram[:],
                    in_offset=bass.IndirectOffsetOnAxis(ap=recvl[:p, :1], axis=0))
                am = sb.tile([P, D], F32R, tag="am", bufs=4, name="am")
                nc.gpsimd.indirect_dma_start(
                    out=am[:p], out_offset=None, in_=amdram[:],
                    in_offset=bass.IndirectOffsetOnAxis(ap=send[:p, :1], axis=0))
                cx["ag"], cx["am"] = ag, am
                recvb = sb.tile([P, 1], F32, tag="recvb", bufs=3, name="recvb")
                nc.sync.dma_start(out=recvb[:p], in_=recvb_t[e0:e0 + p, :])
                cx["recvb"] = recvb
                # eF = e.T ; Ce = e @ W1c
                e_sb = cx.pop("e")
                ptr = ps.tile([P, 4, P], F32R, tag="tr", bufs=2, name="ptr")
                for j in range(4):
                    nc.tensor.transpose(out=ptr[:, j, :p],
                                        in_=e_sb[:p, j * P:(j + 1) * P],
                                        identity=ident[:p, :p])
                eF = sb.tile([P, 4, P], F32R, tag="eF", bufs=3, name="eF")
                nc.vector.tensor_copy(out=eF[:, :, :p], in_=ptr[:, :, :p])
                pce = ps.tile([P, D], F32, tag="mm", bufs=5, name="pce")
                for kt in range(4):
                    nc.tensor.matmul(out=pce[:p], lhsT=eF[:, kt, :p],
                                     rhs=w_ce[:, kt, :], start=(kt == 0), stop=False)
                ag, am = cx.pop("ag"), cx.pop("am")
                u = sb.tile([P, D], F32R, tag="u", bufs=2, name="u")
                nc.vector.tensor_tensor(out=u[:p], in0=ag[:p], in1=am[:p],
                                        op=OP.add)
                nc.tensor.matmul(out=pce[:p], lhsT=ident[:p, :p], rhs=u[:p],
                                 start=False, stop=True)
                cx["pce"] = pce

            def stage3(i):
                b, t0, p = all_tiles[i]
                cx = ctxs[i]
                pce = cx.pop("pce")
                # m1 = relu(Ce + Ag + Am)  (adds already accumulated on PE)
                m1 = sb.tile([P, D], F32R, tag="m1", bufs=2, name="m1")
                nc.scalar.activation(out=m1[:p, :], in_=pce[:p], func=AF.Relu)
                ptr2 = ps.tile([P, 4, P], F32R, tag="tr", bufs=2, name="ptr2")
                for j in range(4):
                    nc.tensor.transpose(out=ptr2[:, j, :p],
                                        in_=m1[:p, j * P:(j + 1) * P],
                                        identity=ident[:p, :p])
                m1F = sb.tile([P, 4, P], F32R, tag="m1F", bufs=3, name="m1F")
                nc.vector.tensor_copy(out=m1F[:, :, :p], in_=ptr2[:, :, :p])
                pg2 = ps.tile([P, D], F32, tag="mm", bufs=5, name="pg2")
                for kt in range(4):
                    nc.tensor.matmul(out=pg2[:p], lhsT=m1F[:, kt, :p],
                                     rhs=w_g2[:, kt, :], start=(kt == 0),
                                     stop=(kt == 3 and not has_b2_g))
                if has_b2_g:
                    nc.tensor.matmul(out=pg2[:p], lhsT=ones_sb[0:1, :p],
                                     rhs=b2g[0:1, :], start=False, stop=True)
                msg = sb.tile([P, D], F32R, tag="msg", bufs=6, name="msg")
                _ln_evac(nc, sb, pg2, msg[:p, :], p, eps_tile, D, "g")
                recvb = cx.pop("recvb")
                oh = sb.tile([P, P], F32R, tag="oh", bufs=6, name="oh")
                nc.vector.tensor_scalar(out=oh[:p, :], in0=iota_f[:p, :],
                                        scalar1=recvb[:p, 0:1], scalar2=None,
                                        op0=OP.is_equal)
                blk_msgs[b].append((msg, oh, p))
                if len(blk_msgs[b]) == T:
                    pending_scatter.append(b)

            def emit_scatter(b):
                pblk = ps.tile([P, D], F32, tag="blk", bufs=1, name="pblk")
                for k, (msg, oh, p) in enumerate(blk_msgs[b]):
                    nc.tensor.matmul(out=pblk[:], lhsT=oh[:p, :], rhs=msg[:p, :],
                                     start=(k == 0), stop=(k == T - 1))
                ab = sb.tile([P, D], F32R, tag="ab", bufs=2, name="ab")
                nc.scalar.copy(out=ab[:], in_=pblk[:])
                ptr3 = ps.tile([P, 4, P], F32R, tag="tr", bufs=2, name="ptr3")
                for j in range(4):
                    nc.tensor.transpose(out=ptr3[:, j, :],
                                        in_=ab[:, j * P:(j + 1) * P],
                                        identity=ident[:])
                abF = sb.tile([P, 4, P], F32R, tag="abF", bufs=2, name="abF")
                nc.vector.tensor_copy(out=abF[:], in_=ptr3[:])
                nc.sync.dma_start(out=afdram[:, :, b * P:(b + 1) * P], in_=abF[:])

            NT = len(all_tiles)
            for i in range(NT + 2):
                if i < NT:
                    stage1(i)
                while pending_scatter:
                    emit_scatter(pending_scatter.pop(0))
                if 0 <= i - 1 < NT:
                    stage2(i - 1)
                if 0 <= i - 2 < NT:
                    stage3(i - 2)
            while pending_scatter:
                emit_scatter(pending_scatter.pop(0))

            # ---------- phase C: node MLPs ----------
            w_n1a = sb.tile([P, 4, D], F32R, tag="w4", bufs=3)
            nc.sync.dma_start(out=w_n1a[:], in_=w_n1_t[:, 0:4, :])
            w_n1b = sb.tile([P, 4, D], F32R, tag="w4", bufs=3)
            nc.sync.dma_start(out=w_n1b[:], in_=w_n1_t[:, 4:8, :])
            w_n2 = sb.tile([P, 4, D], F32R, tag="w4", bufs=3)
            nc.sync.dma_start(out=w_n2[:], in_=w_n2_t[:])
            w_f1a = sb.tile([P, 4, D], F32R, tag="w4b", bufs=3)
            nc.sync.dma_start(out=w_f1a[:], in_=w_f1_t[:, 0:4, :])
            w_f1b = sb.tile([P, 4, D], F32R, tag="w4b", bufs=3)
            nc.sync.dma_start(out=w_f1b[:], in_=w_f1_t[:, 4:8, :])
            w_f2 = sb.tile([P, 4, 256], F32R, tag="w4b", bufs=3)
            nc.sync.dma_start(out=w_f2[:], in_=w_f2_t[:])

            for c in range(NG // CH):
                c0 = c * CH
                xgc = sb.tile([P, 4, CH], F32R, tag="xgc", bufs=2, name="xgc")
                nc.sync.dma_start(out=xgc[:], in_=xgT_t[:, :, c0:c0 + CH])
                afc = sb.tile([P, 4, CH], F32R, tag="afc", bufs=2, name="afc")
                nc.sync.dma_start(out=afc[:], in_=afdram[:, :, c0:c0 + CH])
                # n1F = relu(W_n1.T @ [Xg; aggr])  F-major [512, CH]
                n1F = sb.tile([P, 4, CH], F32R, tag="n1F", bufs=2, name="n1F")
                for m in range(4):
                    pn1 = ps.tile([P, CH], F32, tag="mm", bufs=5, name="pn1")
                    for kt in range(8):
                        lhs = (w_n1a[:, kt, m * P:(m + 1) * P] if kt < 4
                               else w_n1b[:, kt - 4, m * P:(m + 1) * P])
                        rhs = xgc[:, kt, :] if kt < 4 else afc[:, kt - 4, :]
                        nc.tensor.matmul(out=pn1[:], lhsT=lhs, rhs=rhs,
                                         start=(kt == 0),
                                         stop=(kt == 7 and not has_deg))
                    if has_deg:
                        nc.tensor.matmul(out=pn1[:], lhsT=cdeg[0:1, m * P:(m + 1) * P],
                                         rhs=deg_sb[0:1, c0:c0 + CH],
                                         start=False, stop=True)
                    nc.scalar.activation(out=n1F[:, m, :], in_=pn1[:], func=AF.Relu,
                                         bias=b_n1[:, m:m + 1], scale=1.0)
                # per node-tile: n2, LN -> ygn, transpose
                ygnF = sb.tile([P, 4, CH], F32R, tag="ygnF", bufs=2, name="ygnF")
                for ntl in range(CH // P):
                    s0 = ntl * P
                    pn2 = ps.tile([P, D], F32, tag="mm", bufs=5, name="pn2")
                    for kt in range(4):
                        nc.tensor.matmul(out=pn2[:], lhsT=n1F[:, kt, s0:s0 + P],
                                         rhs=w_n2[:, kt, :], start=(kt == 0),
                                         stop=(kt == 3 and not has_b2_n))
                    if has_b2_n:
                        nc.tensor.matmul(out=pn2[:], lhsT=ones_sb[0:1, :P],
                                         rhs=b2n[0:1, :], start=False, stop=True)
                    ygn = sb.tile([P, D], F32R, tag="ygn", bufs=2, name="ygn")
                    _ln_evac(nc, sb, pn2, ygn[:, :], P, eps_tile, D, "n")
                    ptr4 = ps.tile([P, 4, P], F32R, tag="tr", bufs=2, name="ptr4")
                    for j in range(4):
                        nc.tensor.transpose(out=ptr4[:, j, :],
                                            in_=ygn[:, j * P:(j + 1) * P],
                                            identity=ident[:])
                    nc.vector.tensor_copy(out=ygnF[:, :, s0:s0 + P], in_=ptr4[:])
                # f1F = relu(W_f1.T @ [Xg; ygn])
                f1F = sb.tile([P, 4, CH], F32R, tag="f1F", bufs=2, name="f1F")
                for m in range(4):
                    pf1 = ps.tile([P, CH], F32, tag="mm", bufs=5, name="pf1")
                    for kt in range(8):
                        lhs = (w_f1a[:, kt, m * P:(m + 1) * P] if kt < 4
                               else w_f1b[:, kt - 4, m * P:(m + 1) * P])
                        rhs = xgc[:, kt, :] if kt < 4 else ygnF[:, kt - 4, :]
                        nc.tensor.matmul(out=pf1[:], lhsT=lhs, rhs=rhs,
                                         start=(kt == 0), stop=(kt == 7))
                    nc.scalar.activation(out=f1F[:, m, :], in_=pf1[:], func=AF.Relu,
                                         bias=b_f1[:, m:m + 1], scale=1.0)
                # f2 + final LN (+affine) -> out
                for ntl in range(CH // P):
                    s0 = ntl * P
                    pf2 = ps.tile([P, 256], F32, tag="mm", bufs=5, name="pf2")
                    for kt in range(4):
                        nc.tensor.matmul(out=pf2[:], lhsT=f1F[:, kt, s0:s0 + P],
                                         rhs=w_f2[:, kt, :], start=(kt == 0),
                                         stop=(kt == 3 and not has_b2_f))
                    if has_b2_f:
                        nc.tensor.matmul(out=pf2[:], lhsT=ones_sb[0:1, :P],
                                         rhs=b2f[0:1, :], start=False, stop=True)
                    o_sb = sb.tile([P, OUT_DIM], F32, tag="o_sb", bufs=3, name="o_sb")
                    _ln_evac(nc, sb, pf2, o_sb[:, :], P, eps_tile, OUT_DIM, "f")
                    if has_fin_aff:
                        nc.vector.tensor_tensor(out=o_sb[:], in0=o_sb[:],
                                                in1=fing[:], op=OP.mult)
                        nc.vector.tensor_tensor(out=o_sb[:], in0=o_sb[:],
                                                in1=finb[:], op=OP.add)
                    nc.sync.dma_start(out=out_t[c0 + s0:c0 + s0 + P, :], in_=o_sb[:])

    nc.finalize()
    _fix_multi_waits(nc)
    return nc


_cache = {}


def _get_module(c_blk, flags, mm_dt):
    key = (c_blk, flags, str(mm_dt))
    if key not in _cache:
        _cache[key] = _build(c_blk, flags, mm_dt)
    return _cache[key]


def kernel(**inputs):
    _install_ntff_shim()
    f32 = np.float32

    grid = np.asarray(inputs["input_grid_nodes"], f32)
    mesh = np.asarray(inputs["input_mesh_nodes"], f32)
    attr = np.asarray(inputs["input_edge_attr"], f32)
    ei = np.asarray(inputs["edge_index"])
    senders, receivers = ei[0].astype(np.int64), ei[1].astype(np.int64)

    g = {k: np.asarray(v, f32) for k, v in inputs.items()
         if k not in ("input_grid_nodes", "input_mesh_nodes", "input_edge_attr",
                      "edge_index")}

    # ---- fold LN affines / biases into weights (host) ----
    w_h1 = g["emlp_w1"]                        # [4, 512] lhsT
    b_h1v = g["emlp_b1"]                       # per-feature -> evac bias (F-major)
    w_h2 = g["emlp_w2"]                        # [512, 512]
    b2e = g["emlp_b2"]
    ge_w1 = g["ge_w1"]                         # [1536, 512]
    w_ag_m = ge_w1[0:D]
    w_am_m = ge_w1[D:2 * D]
    w_ce_m = g["emlp_g"][:, None] * ge_w1[2 * D:3 * D]
    b1row_v = g["ge_b1"] + g["emlp_beta"] @ ge_w1[2 * D:3 * D]
    w_g2 = g["ge_w2"]
    b2g = g["ge_b2"]
    w_n1_m = np.concatenate([g["gn_w1"][0:D], g["ge_g"][:, None] * g["gn_w1"][D:2 * D]], 0)
    # ge_beta flows through the scatter as deg[n] * (ge_beta @ gn_w1b): rank-1 term
    cdeg_v = g["ge_beta"] @ g["gn_w1"][D:2 * D]
    b_n1v = g["gn_b1"]
    w_n2 = g["gn_w2"]
    b2n = g["gn_b2"]
    w_f1_m = np.concatenate([g["fin_w1"], g["gn_g"][:, None] * g["fin_w1"]], 0)
    b_f1v = g["fin_b1"] + g["gn_beta"] @ g["fin_w1"]
    w_f2 = np.zeros((D, 256), f32)
    w_f2[:, :OUT_DIM] = g["fin_w2"]
    b2f = np.zeros((1, 256), f32)
    b2f[0, :OUT_DIM] = g["fin_b2"]
    fin_g, fin_beta = g["fin_g"], g["fin_beta"]

    flags = (
        bool(np.any(b2e)), bool(np.any(b2g)), bool(np.any(b2n)),
        bool(np.any(g["fin_b2"])), bool(np.any(g["ge_beta"])),
        bool(np.any(fin_beta) or not np.all(fin_g == 1.0)),
        bool(np.any(b1row_v)),
    )

    # ---- shard edges by receiver, per-block padding ----
    perm = np.argsort(receivers, kind="stable")
    r_s, s_s = receivers[perm], senders[perm]
    attr_s = attr[perm]
    blk = (r_s // P).astype(np.int64)                    # global block id 0..127
    bc = np.bincount(blk, minlength=NC_ * NB)
    c_blk = max(P, int(-(-bc.max() // 64)) * 64)
    e_pad = NB * c_blk
    tiles_per_blk = [(t * P, min(P, c_blk - t * P)) for t in range((c_blk + P - 1) // P)]

    # destination slot per edge: block_base + rank within block
    blk_starts = np.zeros(NC_ * NB + 1, np.int64)
    np.cumsum(bc, out=blk_starts[1:])
    rank = np.arange(len(r_s)) - blk_starts[blk]
    core = blk // NB
    slot = (blk % NB) * c_blk + rank                     # slot within core

    send_a = np.zeros((NC_, e_pad, 1), np.int32)
    recvl_a = np.zeros((NC_, e_pad, 1), np.int32)
    recvb_a = np.full((NC_, e_pad, 1), -1.0, f32)
    attr_a = np.zeros((NC_, EDGE_DIM, e_pad), f32)
    send_a[core, slot, 0] = s_s
    recvl_a[core, slot, 0] = r_s - core * NG
    recvb_a[core, slot, 0] = (r_s % P).astype(f32)
    attr_a[core, :, slot] = attr_s

    meshT = np.zeros((D, MESH_PAD), f32)
    meshT[:, :N_MESH] = mesh.T
    deg = np.bincount(receivers, minlength=N_GRID).astype(f32)

    def pack_rhs(w):                                     # [K, N] -> [128, K/128, N]
        return np.ascontiguousarray(w.reshape(-1, P, w.shape[1]).transpose(1, 0, 2))

    def pack_bias(v):                                    # [512] -> [128, 4]
        return np.ascontiguousarray(v.reshape(4, P).T)

    rep = {
        "meshT": np.ascontiguousarray(
            meshT.reshape(4, P, MESH_PAD).transpose(1, 0, 2)),
        "ident": np.eye(P, dtype=f32),
        "ones": np.ones((1, P), f32),
        "w_h1": np.ascontiguousarray(w_h1),
        "b_h1": pack_bias(b_h1v),
        "w_h2": pack_rhs(w_h2),
        "w_ag": pack_rhs(w_ag_m),
        "w_am": pack_rhs(w_am_m),
        "w_ce": pack_rhs(w_ce_m),
        "w_g2": pack_rhs(w_g2),
        "b1row": b1row_v.reshape(1, D).astype(f32),
        "w_n1": pack_rhs(w_n1_m),
        "b_n1": pack_bias(b_n1v),
        "w_n2": pack_rhs(w_n2),
        "w_f1": pack_rhs(w_f1_m),
        "b_f1": pack_bias(b_f1v),
        "w_f2": pack_rhs(w_f2),
        "b2e": b2e.reshape(1, D), "b2g": b2g.reshape(1, D),
        "b2n": b2n.reshape(1, D), "b2f": b2f,
        "cdeg": cdeg_v.reshape(1, D),
        "fing": np.broadcast_to(fin_g, (P, OUT_DIM)).copy(),
        "finb": np.broadcast_to(fin_beta, (P, OUT_DIM)).copy(),
    }

    in_maps = []
    for c in range(NC_):
        xg = grid[c * NG:(c + 1) * NG]
        m = dict(rep)
        m["xgT"] = np.ascontiguousarray(
            xg.T.reshape(4, P, NG).transpose(1, 0, 2))
        m["attrT"] = attr_a[c]
        m["send"] = send_a[c]
        m["recvl"] = recvl_a[c]
        m["recvb"] = recvb_a[c]
        m["deg"] = deg[c * NG:(c + 1) * NG].reshape(1, NG)
        in_maps.append(m)

    use_bf16 = os.environ.get("KERNEL_MM_DT", "f32r") == "bf16"
    mm_dt = mybir.dt.bfloat16 if use_bf16 else mybir.dt.float32r
    if use_bf16:
        import ml_dtypes
        bf16 = ml_dtypes.bfloat16
        mm_keys = ["xgT", "attrT", "meshT", "ident", "ones", "w_h1", "w_h2",
                   "w_ag", "w_am", "w_ce", "w_g2", "b1row", "w_n1", "w_n2",
                   "w_f1", "w_f2", "b2e", "b2g", "b2n", "b2f", "deg", "cdeg"]
        for m in in_maps:
            for k in mm_keys:
                if k in m:
                    m[k] = np.asarray(m[k]).astype(bf16)
    nc = _get_module(c_blk, flags, mm_dt)
    trace = bool(int(os.environ.get("KERNEL_TRACE", "0")))
    res = run_bass_kernel_spmd(nc, in_maps, core_ids=list(range(NC_)), trace=trace)
    if trace:
        kernel.last_exec_time_ns = res.exec_time_ns
        kernel.last_results = res
    return np.concatenate([res.results[c]["out"] for c in range(NC_)], axis=0)
